# revision 29
# baseline (speedup 1.0000x reference)
"""Trainium2 Bass kernel: CrossAttention3D (B=4, Lq=Lk=4096, D=256) on 8 NeuronCores.

Sharding: core c handles batch c//2, decoder-query half c%2 (2048 queries),
with the full encoder sequence for that batch and replicated projections.

v4: K-projection folded away on host (M = Wq @ Wk^T, weight-only), so scores
are (xd @ M) @ xe^T with raw xeT8 slices as the stationary operand. The
Q-side bias term cancels in softmax. When Wk@bq == 0 (always true for this
problem's zero biases) the K-side bias term vanishes too, enabling the fast
path: one full-pair exp per ACT instruction with a constant bias. Otherwise a
half-pair path applies the per-key bias via the ACT bias operand.

Per-core dataflow (all d-major tensors are [128, 2, seq] with d = h*128 + p):
  QT = M^T-chunks (stationary, DR) @ xdT (moving)    -> [d, 2048] fp8
       (PSUM drained by ACT - DVE is busy with V')
  V' = xeT-chunks (stationary) @ [Wvo|u|pad]         -> [k, 256] fp8 + cb f32
  per qc (512 queries), software-pipelined with 2-pair lag:
    st[128k, 2, 512q] = xeT-slices (stationary) @ QT  (two DR matmuls, 2 banks)
    pt = exp(st/16 - 2) fp8                           (one ACT instr per pair)
    O^T[128d, 512q] += V'pair-chunk (stationary) @ pt[p-2]  (two DR matmuls)
    denom[128, 512q] += ones (stationary) @ pt[p-2]         (one DR matmul)
  epilogue: outT = O^T * (1/denom) + (xdT + bo2)   (DVE)
Host: folds Wq@Wk^T, Wv@Wo, bv@Wo+bo, Wk@bq; pre-transposes/casts inputs;
transposes outT back. DMAs ordered so projections start immediately; big
inputs ride separate engine queues to overlap transfers.
"""
import os
import sys

import numpy as np

for _p in ("/opt/trn_rl_repo", os.path.expanduser("~/.axon_site/_ro/trn_rl_repo")):
    if os.path.isdir(_p) and _p not in sys.path:
        sys.path.insert(0, _p)

B, LQ, LK, D = 4, 4096, 4096, 256
NCORES = 8
QCHUNK = LQ // 2          # queries per core
P = 128
SCALE = 1.0 / 16.0        # 1/sqrt(D)
ESHIFT = -2.0             # exp shift (cancels in softmax division)
NQC = QCHUNK // 512       # 4 query chunks of 512 per core
NKT = LK // P             # 32 key tiles
NPAIR = NKT // 2          # 16 key-tile pairs (DoubleRow contracts 256 keys)
VW = 272                  # V' row stride (256 data + 1 cbias + pad, %16==0)
LAG = 3                   # pairs of lag between exp and o-matmul consume
DVE_PAIRS = (8,)          # pairs whose exp runs on DVE (Schraudolph exp2);
                          # late pairs stay on ACT so the chunk boundary
                          # (exp -> o -> o32 drain, next st ring) never waits
                          # on the DVE queue
SCH_A = 12102203.161561485   # 2^23 / ln(2)
SCH_B = 1064986823.0         # 127*2^23 - C, balanced +-3% rel err

_STATE = {}
VARIANT = os.environ.get("KV", "full")


def _build(repeat=1, fastbias=True):
    from contextlib import ExitStack

    import concourse.tile as tile
    from concourse import bacc, mybir

    f32 = mybir.dt.float32
    f8 = mybir.dt.float8e4
    i32 = mybir.dt.int32
    AF = mybir.ActivationFunctionType
    ALU = mybir.AluOpType
    DR = mybir.MatmulPerfMode.DoubleRow

    nc = bacc.Bacc(trn_type="TRN2")
    m8_d = nc.dram_tensor("m8", [P, 2, D], f8, kind="ExternalInput")
    wvu_d = nc.dram_tensor("wvu8", [P, 2, VW], f8, kind="ExternalInput")
    xdT8_d = nc.dram_tensor("xdT8", [P, 2, QCHUNK], f8, kind="ExternalInput")
    xeT8_d = nc.dram_tensor("xeT8", [P, 2, LK], f8, kind="ExternalInput")
    xdT32_d = nc.dram_tensor("xdT32", [P, 2, QCHUNK], f32, kind="ExternalInput")
    outT_d = nc.dram_tensor("outT", [2, P, QCHUNK], f32, kind="ExternalOutput")

    with tile.TileContext(nc) as tc:
        loop_ctx = ExitStack()
        if repeat > 1:
            loop_ctx.enter_context(tc.For_i(0, repeat, 1))
        with (
            tc.tile_pool(name="singles", bufs=1) as singles,
            tc.tile_pool(name="ptp", bufs=LAG + 2) as ptp,
            tc.tile_pool(name="outp", bufs=2) as outp,
            tc.tile_pool(name="recp", bufs=2) as recp,
        ):
            # ---- persistent SBUF tensors; small weights first so the
            # projections can start while the big tensors stream in ----
            m8 = singles.tile([P, 2, D], f8)
            nc.sync.dma_start(out=m8, in_=m8_d[:])
            wvu8 = singles.tile([P, 2, VW], f8)
            nc.sync.dma_start(out=wvu8, in_=wvu_d[:])
            xdT8 = singles.tile([P, 2, QCHUNK], f8)
            nc.sync.dma_start(out=xdT8, in_=xdT8_d[:])
            # xeT8 split: first quarter unblocks early V'/st matmuls while
            # the rest streams in on the ACT hwdge queue
            xeT8 = singles.tile([P, 2, LK], f8)
            nc.scalar.dma_start(out=xeT8[:, :, :1024], in_=xeT8_d[:, :, :1024])
            nc.scalar.dma_start(out=xeT8[:, :, 1024:], in_=xeT8_d[:, :, 1024:])
            xdT32 = singles.tile([P, 2, QCHUNK], f32)
            nc.sync.dma_start(out=xdT32, in_=xdT32_d[:])

            QT8 = singles.tile([P, 2, QCHUNK], f8)
            ones8 = singles.tile([P, 2, P], f8)
            nc.vector.memset(ones8, 1.0)
            expb = singles.tile([P, 1], f32)
            nc.vector.memset(expb, ESHIFT)
            # V' [k_lo, pair, parity, d]; cb2 holds per-key exp bias
            Vp8 = singles.tile([P, NPAIR, 2, VW], f8)
            cb2 = singles.tile([P, NKT], f32)

            # ---------------- projections ----------------
            with (
                tc.tile_pool(name="pj_ps", bufs=2, space="PSUM") as pj_ps,
                tc.tile_pool(name="pv_ps", bufs=3, space="PSUM") as pv_ps,
            ):
                for qc in range(NQC):
                    for c in range(2):
                        pj = pj_ps.tile([P, 512], f32, tag="pj")
                        nc.tensor.matmul(pj, m8[:, :, c * P:(c + 1) * P],
                                         xdT8[:, :, qc * 512:(qc + 1) * 512],
                                         start=True, stop=True, perf_mode=DR)
                        # ACT drains Q' (DVE is busy with V'); exp table not
                        # needed for plain copy
                        nc.scalar.copy(QT8[:, c, qc * 512:(qc + 1) * 512], pj)
                for w in range(NPAIR):
                    # pair of V' matmuls into one 2-bank tile, one DVE drain
                    pv = pv_ps.tile([P, 2, 512], f32, tag="pv")
                    for j in range(2):
                        kt = 2 * w + j
                        nc.tensor.matmul(pv[:, j, :VW],
                                         xeT8[:, :, kt * P:(kt + 1) * P],
                                         wvu8, start=True, stop=True,
                                         perf_mode=DR)
                    nc.vector.tensor_copy(Vp8[:, w, :, 0:D], pv[:, :, :D])
                    if not fastbias:
                        nc.vector.tensor_scalar_add(
                            cb2[:, 2 * w:2 * w + 2], pv[:, :, D:D + 1], expb)

            # ---------------- attention main loop ----------------
            with (
                tc.tile_pool(name="st_ps", bufs=2, space="PSUM") as st_ps,
                tc.tile_pool(name="o_ps", bufs=1, space="PSUM") as o_ps_pool,
                tc.tile_pool(name="dn_ps", bufs=2, space="PSUM") as dn_ps_pool,
            ):
                o_ps = o_ps_pool.tile([P, 2, 512], f32)
                pe_pt = None
                if VARIANT == "pe_only":
                    pe_pt = ptp.tile([P, 2, 512], f8, tag="pt")
                    nc.vector.memset(pe_pt[:, 0:1, 0:1], 0.125)

                def issue_o(p, pt, dn_ps):
                    if VARIANT not in ("no_o", "s_only"):
                        for c in range(2):
                            nc.tensor.matmul(o_ps[:, c, :],
                                             Vp8[:, p, :, c * P:(c + 1) * P],
                                             pt, start=(p == 0),
                                             stop=(p == NPAIR - 1),
                                             perf_mode=DR)
                    if VARIANT not in ("no_dn", "s_only"):
                        nc.tensor.matmul(dn_ps, ones8, pt,
                                         start=(p == 0),
                                         stop=(p == NPAIR - 1),
                                         perf_mode=DR)

                def epilogue(qc, dn_ps):
                    qsl = slice(qc * 512, (qc + 1) * 512)
                    # drain O^T out of PSUM (frees banks for next qc asap)
                    o32 = outp.tile([P, 2, 512], f32, tag="o32")
                    if VARIANT in ("s_only", "no_o"):
                        nc.vector.memset(o32, 1.0)
                    else:
                        nc.vector.tensor_copy(o32, o_ps)
                    # divide by denom, add residual(+bo2), store
                    recB = recp.tile([P, 512], f32, tag="recB")
                    if VARIANT in ("no_dn", "s_only"):
                        nc.vector.memset(recB, 1.0)
                    else:
                        nc.vector.reciprocal(recB, dn_ps)
                    for c in range(2):
                        outf = outp.tile([P, 512], f32, tag="outf")
                        nc.vector.tensor_mul(outf, o32[:, c, :], recB)
                        nc.vector.tensor_add(outf, outf, xdT32[:, c, qsl])
                        nc.sync.dma_start(out=outT_d[c, :, qsl], in_=outf)

                # flat pipeline over all (qc, pair) slots: next chunk's st
                # matmuls fill the o-matmul tail of the previous chunk
                NTOT = NQC * NPAIR
                pts = {}
                dns = {}
                for s in range(NTOT + LAG):
                    if s < NTOT:
                        qc, p = divmod(s, NPAIR)
                        qsl = slice(qc * 512, (qc + 1) * 512)
                        st = st_ps.tile([P, 2, 512], f32, tag="st")
                        for j in range(2):
                            kt = 2 * p + j
                            nc.tensor.matmul(st[:, j, :],
                                             xeT8[:, :, kt * P:(kt + 1) * P],
                                             QT8[:, :, qsl],
                                             start=True, stop=True,
                                             perf_mode=DR)
                        if VARIANT == "pe_only":
                            pts[s] = pe_pt
                        else:
                            pts[s] = ptp.tile([P, 2, 512], f8, tag="pt",
                                              name=f"pt{s}")
                            if fastbias and p in DVE_PAIRS:
                                # Schraudolph exp2 on DVE: int32 bits of
                                # A*(scale*st+shift)+B reinterpreted as f32
                                # approximate exp to +-3%, unloading the ACT
                                # critical path
                                ti = ptp.tile([P, 2, 512], i32, tag="ti",
                                              name=f"ti{s}")
                                nc.vector.tensor_scalar(
                                    ti, st, SCH_A * SCALE,
                                    SCH_A * ESHIFT + SCH_B,
                                    op0=ALU.mult, op1=ALU.add)
                                nc.vector.tensor_copy(
                                    pts[s], ti[:].bitcast(f32))
                            elif fastbias:
                                nc.scalar.activation(pts[s], st, AF.Exp,
                                                     bias=expb, scale=SCALE)
                            else:
                                for j in range(2):
                                    kt = 2 * p + j
                                    nc.scalar.activation(
                                        pts[s][:, j, :], st[:, j, :],
                                        AF.Exp, bias=cb2[:, kt:kt + 1],
                                        scale=SCALE)
                    so = s - LAG
                    if 0 <= so < NTOT:
                        qco, po = divmod(so, NPAIR)
                        if po == 0:
                            dns[qco] = dn_ps_pool.tile([P, 512], f32,
                                                       tag="dn",
                                                       name=f"dn{qco}")
                        issue_o(po, pts.pop(so), dns[qco])
                        if po == NPAIR - 1:
                            epilogue(qco, dns.pop(qco))

        loop_ctx.close()

    nc.finalize()
    return nc


def _get_nc(repeat=1, fastbias=True):
    key = f"nc{repeat}_{fastbias}"
    if key not in _STATE:
        _STATE[key] = _build(repeat, fastbias)
    return _STATE[key]


def _dmajor(a):
    """[d(256), n] f32 -> [128, 2, n] contiguous (d = h*128 + p)."""
    n = a.shape[1]
    return np.ascontiguousarray(a.reshape(2, P, n).transpose(1, 0, 2))


def _in_maps(x_decoder, x_encoder, Wq, bq, Wk, bk, Wv, bv, Wo, bo):
    import ml_dtypes
    f8 = ml_dtypes.float8_e4m3

    x_decoder = np.asarray(x_decoder, dtype=np.float32)
    x_encoder = np.asarray(x_encoder, dtype=np.float32)
    Wq, Wk, Wv, Wo = (np.asarray(w, dtype=np.float64) for w in (Wq, Wk, Wv, Wo))
    bq, bk, bv, bo = (np.asarray(b, dtype=np.float64) for b in (bq, bk, bv, bo))
    bo2 = (bv @ Wo + bo).astype(np.float32)
    Wvo = (Wv @ Wo).astype(np.float32)
    M = (Wq @ Wk.T).astype(np.float32)
    # K-side bias term: scores include (xe_k . u) per key (the Q-side
    # counterpart and the bq.bk constant cancel in softmax). u pre-scaled so
    # the ACT bias is a pure add. Zero u (always, given zero biases) enables
    # the constant-bias fast path.
    u = (SCALE * (Wk @ bq)).astype(np.float32)
    _STATE["fastbias"] = bool(np.all(u == 0.0))

    m8 = np.ascontiguousarray(M.reshape(2, P, D).transpose(1, 0, 2)).astype(f8)
    wvu = np.zeros((D, VW), dtype=np.float32)
    wvu[:, :D] = Wvo
    wvu[:, D] = u
    wvu8 = np.ascontiguousarray(
        wvu.reshape(2, P, VW).transpose(1, 0, 2)).astype(f8)

    maps = []
    for c in range(NCORES):
        b, h = divmod(c, 2)
        xd = x_decoder[b, h * QCHUNK:(h + 1) * QCHUNK]        # [2048, 256]
        xdT = np.ascontiguousarray(xd.T)                      # [256, 2048]
        xeT = np.ascontiguousarray(x_encoder[b].T)            # [256, 4096]
        maps.append({
            "xdT8": _dmajor(xdT).astype(f8),
            "xdT32": _dmajor(xdT + bo2[:, None]),
            "xeT8": _dmajor(xeT).astype(f8),
            "m8": m8, "wvu8": wvu8,
        })
    return maps


def _assemble(results):
    out = np.empty((B, LQ, D), dtype=np.float32)
    for c in range(NCORES):
        b, h = divmod(c, 2)
        outT = results[c]["outT"].reshape(D, QCHUNK)
        out[b, h * QCHUNK:(h + 1) * QCHUNK] = outT.T
    return out


def _get_compiled(repeat=1):
    """Build a reusable jitted SPMD executable (compiles once per process)."""
    fastbias = _STATE.get("fastbias", True)
    ckey = f"compiled{repeat}_{fastbias}"
    if ckey in _STATE:
        return _STATE[ckey]
    import jax
    import numpy as jnp_np
    from jax.sharding import Mesh, PartitionSpec
    from jax.experimental.shard_map import shard_map
    from concourse import bass2jax, mybir

    nc = _get_nc(repeat, fastbias)
    bass2jax.install_neuronx_cc_hook()
    partition_name = (nc.partition_id_tensor.name
                      if nc.partition_id_tensor else None)
    in_names, out_names, out_avals, zero_outs = [], [], [], []
    for alloc in nc.m.functions[0].allocations:
        if not isinstance(alloc, mybir.MemoryLocationSet):
            continue
        name = alloc.memorylocations[0].name
        if alloc.kind == "ExternalInput":
            if name != partition_name:
                in_names.append(name)
        elif alloc.kind == "ExternalOutput":
            shape = tuple(alloc.tensor_shape)
            dtype = mybir.dt.np(alloc.dtype)
            out_names.append(name)
            out_avals.append(jax.core.ShapedArray(shape, dtype))
            zero_outs.append(np.zeros((NCORES * shape[0], *shape[1:]), dtype))
    n_params = len(in_names)
    all_names = in_names + out_names
    if partition_name is not None:
        all_names.append(partition_name)

    def _body(*args):
        operands = list(args)
        if partition_name is not None:
            operands.append(bass2jax.partition_id_tensor())
        outs = bass2jax._bass_exec_p.bind(
            *operands,
            out_avals=tuple(out_avals),
            in_names=tuple(all_names),
            out_names=tuple(out_names),
            lowering_input_output_aliases=(),
            sim_require_finite=True,
            sim_require_nnan=True,
            nc=nc,
        )
        return tuple(outs)

    devices = jax.devices()[:NCORES]
    mesh = Mesh(jnp_np.asarray(devices), ("core",))
    nio = n_params + len(out_names)
    sharded = jax.jit(
        shard_map(_body, mesh=mesh,
                  in_specs=(PartitionSpec("core"),) * nio,
                  out_specs=(PartitionSpec("core"),) * len(out_names),
                  check_rep=False),
        keep_unused=True,
    )
    _STATE[ckey] = (sharded, in_names, out_names, out_avals, zero_outs, mesh)
    return _STATE[ckey]


def _concat_inputs(maps, in_names):
    return [np.concatenate([maps[c][n] for c in range(NCORES)], axis=0)
            for n in in_names]


def run_maps(maps):
    sharded, in_names, out_names, out_avals, zero_outs, mesh = _get_compiled()
    concat_in = _concat_inputs(maps, in_names)
    out_arrs = sharded(*concat_in, *zero_outs)
    results = []
    for c in range(NCORES):
        results.append({
            name: np.asarray(out_arrs[i]).reshape(NCORES, *out_avals[i].shape)[c]
            for i, name in enumerate(out_names)})
    return results


def kernel(x_decoder, x_encoder, Wq, bq, Wk, bk, Wv, bv, Wo, bo):
    maps = _in_maps(x_decoder, x_encoder, Wq, bq, Wk, bk, Wv, bv, Wo, bo)
    return _assemble(run_maps(maps))


def bench(maps, iters=30, repeat=1):
    """Time repeated executions with device-resident inputs; returns seconds/iter."""
    import time

    import jax
    from jax.sharding import NamedSharding, PartitionSpec

    sharded, in_names, out_names, out_avals, zero_outs, mesh = _get_compiled(repeat)
    sh = NamedSharding(mesh, PartitionSpec("core"))
    dev_in = [jax.device_put(a, sh) for a in _concat_inputs(maps, in_names)]
    dev_zero = [jax.device_put(z, sh) for z in zero_outs]
    jax.block_until_ready(dev_in + dev_zero)
    out = sharded(*dev_in, *dev_zero)
    jax.block_until_ready(out)
    times = []
    for _ in range(iters):
        t0 = time.perf_counter()
        out = sharded(*dev_in, *dev_zero)
        jax.block_until_ready(out)
        times.append(time.perf_counter() - t0)
    times.sort()
    return {"min": times[0], "median": times[len(times) // 2],
            "mean": sum(times) / len(times)}


# revision 30
# speedup vs baseline: 1.0395x; 1.0395x over previous
"""Trainium2 Bass kernel: CrossAttention3D (B=4, Lq=Lk=4096, D=256) on 8 NeuronCores.

Sharding: core c handles batch c//2, decoder-query half c%2 (2048 queries),
with the full encoder sequence for that batch and replicated projections.

v4: K-projection folded away on host (M = Wq @ Wk^T, weight-only), so scores
are (xd @ M) @ xe^T with raw xeT8 slices as the stationary operand. The
Q-side bias term cancels in softmax. When Wk@bq == 0 (always true for this
problem's zero biases) the K-side bias term vanishes too, enabling the fast
path: one full-pair exp per ACT instruction with a constant bias. Otherwise a
half-pair path applies the per-key bias via the ACT bias operand.

Per-core dataflow (all d-major tensors are [128, 2, seq] with d = h*128 + p):
  QT = M^T-chunks (stationary, DR) @ xdT (moving)    -> [d, 2048] fp8
       (PSUM drained by ACT - DVE is busy with V')
  V' = xeT-chunks (stationary) @ [Wvo|u|pad]         -> [k, 256] fp8 + cb f32
  per qc (512 queries), software-pipelined with 2-pair lag:
    st[128k, 2, 512q] = xeT-slices (stationary) @ QT  (two DR matmuls, 2 banks)
    pt = exp(st/16 - 2) fp8                           (one ACT instr per pair)
    O^T[128d, 512q] += V'pair-chunk (stationary) @ pt[p-2]  (two DR matmuls)
    denom[128, 512q] += ones (stationary) @ pt[p-2]         (one DR matmul)
  epilogue: outT = O^T * (1/denom) + (xdT + bo2)   (DVE)
Host: folds Wq@Wk^T, Wv@Wo, bv@Wo+bo, Wk@bq; pre-transposes/casts inputs;
transposes outT back. DMAs ordered so projections start immediately; big
inputs ride separate engine queues to overlap transfers.
"""
import os
import sys

import numpy as np

for _p in ("/opt/trn_rl_repo", os.path.expanduser("~/.axon_site/_ro/trn_rl_repo")):
    if os.path.isdir(_p) and _p not in sys.path:
        sys.path.insert(0, _p)

B, LQ, LK, D = 4, 4096, 4096, 256
NCORES = 8
QCHUNK = LQ // 2          # queries per core
P = 128
SCALE = 1.0 / 16.0        # 1/sqrt(D)
ESHIFT = -2.0             # exp shift (cancels in softmax division)
NQC = QCHUNK // 512       # 4 query chunks of 512 per core
NKT = LK // P             # 32 key tiles
NPAIR = NKT // 2          # 16 key-tile pairs (DoubleRow contracts 256 keys)
VW = 272                  # V' row stride (256 data + 1 cbias + pad, %16==0)
LAG = 3                   # pairs of lag between exp and o-matmul consume
DVE_PAIRS = (8, 14)       # pairs whose exp runs on DVE (Schraudolph exp2);
                          # pair 15 stays on ACT so the chunk-boundary chain
                          # (exp15 -> o(15) -> o32 drain) never waits on DVE
SCH_A = 12102203.161561485   # 2^23 / ln(2)
SCH_B = 1064986823.0         # 127*2^23 - C, balanced +-3% rel err

_STATE = {}
VARIANT = os.environ.get("KV", "full")


def _build(repeat=1, fastbias=True):
    from contextlib import ExitStack

    import concourse.tile as tile
    from concourse import bacc, mybir

    f32 = mybir.dt.float32
    f8 = mybir.dt.float8e4
    i32 = mybir.dt.int32
    AF = mybir.ActivationFunctionType
    ALU = mybir.AluOpType
    DR = mybir.MatmulPerfMode.DoubleRow

    nc = bacc.Bacc(trn_type="TRN2")
    m8_d = nc.dram_tensor("m8", [P, 2, D], f8, kind="ExternalInput")
    wvu_d = nc.dram_tensor("wvu8", [P, 2, VW], f8, kind="ExternalInput")
    xdT8_d = nc.dram_tensor("xdT8", [P, 2, QCHUNK], f8, kind="ExternalInput")
    xeT8_d = nc.dram_tensor("xeT8", [P, 2, LK], f8, kind="ExternalInput")
    xdT32_d = nc.dram_tensor("xdT32", [P, 2, QCHUNK], f32, kind="ExternalInput")
    outT_d = nc.dram_tensor("outT", [2, P, QCHUNK], f32, kind="ExternalOutput")

    with tile.TileContext(nc) as tc:
        loop_ctx = ExitStack()
        if repeat > 1:
            loop_ctx.enter_context(tc.For_i(0, repeat, 1))
        with (
            tc.tile_pool(name="singles", bufs=1) as singles,
            tc.tile_pool(name="ptp", bufs=LAG + 2) as ptp,
            tc.tile_pool(name="outp", bufs=2) as outp,
            tc.tile_pool(name="recp", bufs=2) as recp,
        ):
            # ---- persistent SBUF tensors; small weights first so the
            # projections can start while the big tensors stream in ----
            m8 = singles.tile([P, 2, D], f8)
            nc.sync.dma_start(out=m8, in_=m8_d[:])
            wvu8 = singles.tile([P, 2, VW], f8)
            nc.sync.dma_start(out=wvu8, in_=wvu_d[:])
            xdT8 = singles.tile([P, 2, QCHUNK], f8)
            nc.sync.dma_start(out=xdT8, in_=xdT8_d[:])
            # xeT8 split: first quarter unblocks early V'/st matmuls while
            # the rest streams in on the ACT hwdge queue
            xeT8 = singles.tile([P, 2, LK], f8)
            nc.scalar.dma_start(out=xeT8[:, :, :1024], in_=xeT8_d[:, :, :1024])
            nc.scalar.dma_start(out=xeT8[:, :, 1024:], in_=xeT8_d[:, :, 1024:])
            xdT32 = singles.tile([P, 2, QCHUNK], f32)
            nc.sync.dma_start(out=xdT32, in_=xdT32_d[:])

            QT8 = singles.tile([P, 2, QCHUNK], f8)
            ones8 = singles.tile([P, 2, P], f8)
            nc.vector.memset(ones8, 1.0)
            expb = singles.tile([P, 1], f32)
            nc.vector.memset(expb, ESHIFT)
            # V' [k_lo, pair, parity, d]; cb2 holds per-key exp bias
            Vp8 = singles.tile([P, NPAIR, 2, VW], f8)
            cb2 = singles.tile([P, NKT], f32)

            # ---------------- projections ----------------
            with (
                tc.tile_pool(name="pj_ps", bufs=2, space="PSUM") as pj_ps,
                tc.tile_pool(name="pv_ps", bufs=3, space="PSUM") as pv_ps,
            ):
                for qc in range(NQC):
                    for c in range(2):
                        pj = pj_ps.tile([P, 512], f32, tag="pj")
                        nc.tensor.matmul(pj, m8[:, :, c * P:(c + 1) * P],
                                         xdT8[:, :, qc * 512:(qc + 1) * 512],
                                         start=True, stop=True, perf_mode=DR)
                        # ACT drains Q' (DVE is busy with V'); exp table not
                        # needed for plain copy
                        nc.scalar.copy(QT8[:, c, qc * 512:(qc + 1) * 512], pj)
                for w in range(NPAIR):
                    # pair of V' matmuls into one 2-bank tile, one DVE drain
                    pv = pv_ps.tile([P, 2, 512], f32, tag="pv")
                    for j in range(2):
                        kt = 2 * w + j
                        nc.tensor.matmul(pv[:, j, :VW],
                                         xeT8[:, :, kt * P:(kt + 1) * P],
                                         wvu8, start=True, stop=True,
                                         perf_mode=DR)
                    nc.vector.tensor_copy(Vp8[:, w, :, 0:D], pv[:, :, :D])
                    if not fastbias:
                        nc.vector.tensor_scalar_add(
                            cb2[:, 2 * w:2 * w + 2], pv[:, :, D:D + 1], expb)

            # ---------------- attention main loop ----------------
            with (
                tc.tile_pool(name="st_ps", bufs=2, space="PSUM") as st_ps,
                tc.tile_pool(name="o_ps", bufs=1, space="PSUM") as o_ps_pool,
                tc.tile_pool(name="dn_ps", bufs=2, space="PSUM") as dn_ps_pool,
            ):
                o_ps = o_ps_pool.tile([P, 2, 512], f32)
                pe_pt = None
                if VARIANT == "pe_only":
                    pe_pt = ptp.tile([P, 2, 512], f8, tag="pt")
                    nc.vector.memset(pe_pt[:, 0:1, 0:1], 0.125)

                def issue_o(p, pt, dn_ps):
                    if VARIANT not in ("no_o", "s_only"):
                        for c in range(2):
                            nc.tensor.matmul(o_ps[:, c, :],
                                             Vp8[:, p, :, c * P:(c + 1) * P],
                                             pt, start=(p == 0),
                                             stop=(p == NPAIR - 1),
                                             perf_mode=DR)
                    if VARIANT not in ("no_dn", "s_only"):
                        nc.tensor.matmul(dn_ps, ones8, pt,
                                         start=(p == 0),
                                         stop=(p == NPAIR - 1),
                                         perf_mode=DR)

                def epilogue(qc, dn_ps):
                    qsl = slice(qc * 512, (qc + 1) * 512)
                    # drain O^T out of PSUM (frees banks for next qc asap)
                    o32 = outp.tile([P, 2, 512], f32, tag="o32")
                    if VARIANT in ("s_only", "no_o"):
                        nc.vector.memset(o32, 1.0)
                    else:
                        nc.vector.tensor_copy(o32, o_ps)
                    # divide by denom, add residual(+bo2), store
                    recB = recp.tile([P, 512], f32, tag="recB")
                    if VARIANT in ("no_dn", "s_only"):
                        nc.vector.memset(recB, 1.0)
                    else:
                        nc.vector.reciprocal(recB, dn_ps)
                    for c in range(2):
                        outf = outp.tile([P, 512], f32, tag="outf")
                        nc.vector.tensor_mul(outf, o32[:, c, :], recB)
                        nc.vector.tensor_add(outf, outf, xdT32[:, c, qsl])
                        nc.sync.dma_start(out=outT_d[c, :, qsl], in_=outf)

                # flat pipeline over all (qc, pair) slots: next chunk's st
                # matmuls fill the o-matmul tail of the previous chunk
                NTOT = NQC * NPAIR
                pts = {}
                dns = {}
                for s in range(NTOT + LAG):
                    if s < NTOT:
                        qc, p = divmod(s, NPAIR)
                        qsl = slice(qc * 512, (qc + 1) * 512)
                        st = st_ps.tile([P, 2, 512], f32, tag="st")
                        for j in range(2):
                            kt = 2 * p + j
                            nc.tensor.matmul(st[:, j, :],
                                             xeT8[:, :, kt * P:(kt + 1) * P],
                                             QT8[:, :, qsl],
                                             start=True, stop=True,
                                             perf_mode=DR)
                        if VARIANT == "pe_only":
                            pts[s] = pe_pt
                        else:
                            pts[s] = ptp.tile([P, 2, 512], f8, tag="pt",
                                              name=f"pt{s}")
                            if fastbias and p in DVE_PAIRS:
                                # Schraudolph exp2 on DVE: int32 bits of
                                # A*(scale*st+shift)+B reinterpreted as f32
                                # approximate exp to +-3%, unloading the ACT
                                # critical path
                                ti = ptp.tile([P, 2, 512], i32, tag="ti",
                                              name=f"ti{s}")
                                nc.vector.tensor_scalar(
                                    ti, st, SCH_A * SCALE,
                                    SCH_A * ESHIFT + SCH_B,
                                    op0=ALU.mult, op1=ALU.add)
                                nc.vector.tensor_copy(
                                    pts[s], ti[:].bitcast(f32))
                            elif fastbias:
                                nc.scalar.activation(pts[s], st, AF.Exp,
                                                     bias=expb, scale=SCALE)
                            else:
                                for j in range(2):
                                    kt = 2 * p + j
                                    nc.scalar.activation(
                                        pts[s][:, j, :], st[:, j, :],
                                        AF.Exp, bias=cb2[:, kt:kt + 1],
                                        scale=SCALE)
                    so = s - LAG
                    if 0 <= so < NTOT:
                        qco, po = divmod(so, NPAIR)
                        if po == 0:
                            dns[qco] = dn_ps_pool.tile([P, 512], f32,
                                                       tag="dn",
                                                       name=f"dn{qco}")
                        issue_o(po, pts.pop(so), dns[qco])
                        if po == NPAIR - 1:
                            epilogue(qco, dns.pop(qco))

        loop_ctx.close()

    nc.finalize()
    return nc


def _get_nc(repeat=1, fastbias=True):
    key = f"nc{repeat}_{fastbias}"
    if key not in _STATE:
        _STATE[key] = _build(repeat, fastbias)
    return _STATE[key]


def _dmajor(a):
    """[d(256), n] f32 -> [128, 2, n] contiguous (d = h*128 + p)."""
    n = a.shape[1]
    return np.ascontiguousarray(a.reshape(2, P, n).transpose(1, 0, 2))


def _in_maps(x_decoder, x_encoder, Wq, bq, Wk, bk, Wv, bv, Wo, bo):
    import ml_dtypes
    f8 = ml_dtypes.float8_e4m3

    x_decoder = np.asarray(x_decoder, dtype=np.float32)
    x_encoder = np.asarray(x_encoder, dtype=np.float32)
    Wq, Wk, Wv, Wo = (np.asarray(w, dtype=np.float64) for w in (Wq, Wk, Wv, Wo))
    bq, bk, bv, bo = (np.asarray(b, dtype=np.float64) for b in (bq, bk, bv, bo))
    bo2 = (bv @ Wo + bo).astype(np.float32)
    Wvo = (Wv @ Wo).astype(np.float32)
    M = (Wq @ Wk.T).astype(np.float32)
    # K-side bias term: scores include (xe_k . u) per key (the Q-side
    # counterpart and the bq.bk constant cancel in softmax). u pre-scaled so
    # the ACT bias is a pure add. Zero u (always, given zero biases) enables
    # the constant-bias fast path.
    u = (SCALE * (Wk @ bq)).astype(np.float32)
    _STATE["fastbias"] = bool(np.all(u == 0.0))

    m8 = np.ascontiguousarray(M.reshape(2, P, D).transpose(1, 0, 2)).astype(f8)
    wvu = np.zeros((D, VW), dtype=np.float32)
    wvu[:, :D] = Wvo
    wvu[:, D] = u
    wvu8 = np.ascontiguousarray(
        wvu.reshape(2, P, VW).transpose(1, 0, 2)).astype(f8)

    maps = []
    for c in range(NCORES):
        b, h = divmod(c, 2)
        xd = x_decoder[b, h * QCHUNK:(h + 1) * QCHUNK]        # [2048, 256]
        xdT = np.ascontiguousarray(xd.T)                      # [256, 2048]
        xeT = np.ascontiguousarray(x_encoder[b].T)            # [256, 4096]
        maps.append({
            "xdT8": _dmajor(xdT).astype(f8),
            "xdT32": _dmajor(xdT + bo2[:, None]),
            "xeT8": _dmajor(xeT).astype(f8),
            "m8": m8, "wvu8": wvu8,
        })
    return maps


def _assemble(results):
    out = np.empty((B, LQ, D), dtype=np.float32)
    for c in range(NCORES):
        b, h = divmod(c, 2)
        outT = results[c]["outT"].reshape(D, QCHUNK)
        out[b, h * QCHUNK:(h + 1) * QCHUNK] = outT.T
    return out


def _get_compiled(repeat=1):
    """Build a reusable jitted SPMD executable (compiles once per process)."""
    fastbias = _STATE.get("fastbias", True)
    ckey = f"compiled{repeat}_{fastbias}"
    if ckey in _STATE:
        return _STATE[ckey]
    import jax
    import numpy as jnp_np
    from jax.sharding import Mesh, PartitionSpec
    from jax.experimental.shard_map import shard_map
    from concourse import bass2jax, mybir

    nc = _get_nc(repeat, fastbias)
    bass2jax.install_neuronx_cc_hook()
    partition_name = (nc.partition_id_tensor.name
                      if nc.partition_id_tensor else None)
    in_names, out_names, out_avals, zero_outs = [], [], [], []
    for alloc in nc.m.functions[0].allocations:
        if not isinstance(alloc, mybir.MemoryLocationSet):
            continue
        name = alloc.memorylocations[0].name
        if alloc.kind == "ExternalInput":
            if name != partition_name:
                in_names.append(name)
        elif alloc.kind == "ExternalOutput":
            shape = tuple(alloc.tensor_shape)
            dtype = mybir.dt.np(alloc.dtype)
            out_names.append(name)
            out_avals.append(jax.core.ShapedArray(shape, dtype))
            zero_outs.append(np.zeros((NCORES * shape[0], *shape[1:]), dtype))
    n_params = len(in_names)
    all_names = in_names + out_names
    if partition_name is not None:
        all_names.append(partition_name)

    def _body(*args):
        operands = list(args)
        if partition_name is not None:
            operands.append(bass2jax.partition_id_tensor())
        outs = bass2jax._bass_exec_p.bind(
            *operands,
            out_avals=tuple(out_avals),
            in_names=tuple(all_names),
            out_names=tuple(out_names),
            lowering_input_output_aliases=(),
            sim_require_finite=True,
            sim_require_nnan=True,
            nc=nc,
        )
        return tuple(outs)

    devices = jax.devices()[:NCORES]
    mesh = Mesh(jnp_np.asarray(devices), ("core",))
    nio = n_params + len(out_names)
    sharded = jax.jit(
        shard_map(_body, mesh=mesh,
                  in_specs=(PartitionSpec("core"),) * nio,
                  out_specs=(PartitionSpec("core"),) * len(out_names),
                  check_rep=False),
        keep_unused=True,
    )
    _STATE[ckey] = (sharded, in_names, out_names, out_avals, zero_outs, mesh)
    return _STATE[ckey]


def _concat_inputs(maps, in_names):
    return [np.concatenate([maps[c][n] for c in range(NCORES)], axis=0)
            for n in in_names]


def run_maps(maps):
    sharded, in_names, out_names, out_avals, zero_outs, mesh = _get_compiled()
    concat_in = _concat_inputs(maps, in_names)
    out_arrs = sharded(*concat_in, *zero_outs)
    results = []
    for c in range(NCORES):
        results.append({
            name: np.asarray(out_arrs[i]).reshape(NCORES, *out_avals[i].shape)[c]
            for i, name in enumerate(out_names)})
    return results


def kernel(x_decoder, x_encoder, Wq, bq, Wk, bk, Wv, bv, Wo, bo):
    maps = _in_maps(x_decoder, x_encoder, Wq, bq, Wk, bk, Wv, bv, Wo, bo)
    return _assemble(run_maps(maps))


def bench(maps, iters=30, repeat=1):
    """Time repeated executions with device-resident inputs; returns seconds/iter."""
    import time

    import jax
    from jax.sharding import NamedSharding, PartitionSpec

    sharded, in_names, out_names, out_avals, zero_outs, mesh = _get_compiled(repeat)
    sh = NamedSharding(mesh, PartitionSpec("core"))
    dev_in = [jax.device_put(a, sh) for a in _concat_inputs(maps, in_names)]
    dev_zero = [jax.device_put(z, sh) for z in zero_outs]
    jax.block_until_ready(dev_in + dev_zero)
    out = sharded(*dev_in, *dev_zero)
    jax.block_until_ready(out)
    times = []
    for _ in range(iters):
        t0 = time.perf_counter()
        out = sharded(*dev_in, *dev_zero)
        jax.block_until_ready(out)
        times.append(time.perf_counter() - t0)
    times.sort()
    return {"min": times[0], "median": times[len(times) // 2],
            "mean": sum(times) / len(times)}
